# revision 2
# baseline (speedup 1.0000x reference)
"""Trainium2 Bass kernel for BuiltSWAP: out = (state_re + i*state_im) @ M.

Fast path (the BuiltSWAP case): M is exactly the SWAP-gate permutation matrix
for qubits (a=0, b=7) on 13 qubits, i.e. out[:, j] = state[:, swap_bits(j)]
where swap_bits exchanges bits 12 and 5 of the column index.  The matmul is
then pure data movement, so the kernel is a DMA-only permutation copy:

  - Layout: state packed column-major as S[8192, 128] (64 re + 64 im lanes),
    bf16 (rel err <= 2^-8 ~ 4e-3, well inside the 2e-2 gate).
  - Shard output columns j across 8 cores (1024 each).  Core c (X=c>>2 the
    fixed bit12 of its j-range, B=c&3 bits 11..10) needs input rows
    y*4096 + B*1024 + h*64 + X*32 + l  for y in {0,1}, h in {0..15},
    l in {0..31} — staged per-core as I_c[y, h, l, 128] (a pure strided
    slice of S; 256KB).
  - Device: one DRAM->DRAM DMA per core with a transposed access pattern
    out[h, y, l, :] = I_c[y, h, l, :] — the y<->h transpose IS the
    bit12<->bit5 swap at 8KB-contiguous-chunk granularity.
  - Per-core HBM traffic: 256KB read + 256KB write per invocation.

Fallback (arbitrary M): dense matmul, fp16 hi/lo state x fp8/bf16 M,
column-sharded over 8 cores (~52us).
"""

import numpy as np
import ml_dtypes

BATCH = 64
N = 8192
NCORES = 8
P = 128

f8e4 = ml_dtypes.float8_e4m3
bf16 = ml_dtypes.bfloat16

_cached = {}

# ---------------------------------------------------------------------------
# Fast path: SWAP permutation as a DMA copy
# ---------------------------------------------------------------------------

SWAP_MASK = (1 << 12) | (1 << 5)  # qubits (0,7) on 13 qubits, bit-flipped


def _swap_perm():
    i = np.arange(N)
    differ = ((i >> 12) & 1) != ((i >> 5) & 1)
    return np.where(differ, i ^ SWAP_MASK, i)


def _is_swap_matrix(M):
    # exact check that M is the expected permutation matrix: its 8192
    # nonzeros are 1.0 at rows perm[j] for column j.
    if M.shape != (N, N):
        return False
    perm = _swap_perm()
    if not np.all(M[perm, np.arange(N)] == 1.0):
        return False
    return np.count_nonzero(M) == N


def _build_perm_program(reps=1, serialize=False, mode="d2d"):
    # reps>1 repeats the permutation copy inside one NEFF (for benchmarking),
    # rotating over 4 output buffers so reps pipeline; serialize adds an
    # all-engine barrier between reps to approximate single-shot cost.
    import concourse.mybir as mybir
    import concourse.tile as tile
    from concourse import bacc

    nc = bacc.Bacc("TRN2", target_bir_lowering=False, debug=False)
    in_d = nc.declare_dram_parameter(
        "in", [2, 16, 4, 8, 128], mybir.dt.bfloat16, isOutput=False
    )
    nout = 4 if reps > 1 else 1
    outs = [
        nc.declare_dram_parameter(
            f"out{i}", [16, 2, 4, 8, 128], mybir.dt.bfloat16, isOutput=True
        )
        for i in range(nout)
    ]
    src = in_d[:, :, :, :, :].rearrange("y h c w b -> h y c w b")
    with tile.TileContext(nc) as tc:
        if mode == "d2d":
            for r in range(reps):
                if serialize and reps > 1 and r > 0:
                    tc.strict_bb_all_engine_barrier()
                nc.sync.dma_start(outs[r % nout][:, :, :, :, :], src)
        else:  # through SBUF, double buffered
            with tc.tile_pool(name="p", bufs=2) as pool:
                for r in range(reps):
                    if serialize and reps > 1 and r > 0:
                        tc.strict_bb_all_engine_barrier()
                    t = pool.tile([P, 8, 128], mybir.dt.bfloat16, name="t")
                    nc.sync.dma_start(t[:, :, :], src)
                    nc.sync.dma_start(outs[r % nout][:, :, :, :, :], t[:, :, :])
    nc.compile()
    return nc


def _get_perm_program(mode="d2d"):
    key = f"perm_{mode}"
    if key not in _cached:
        _cached[key] = _build_perm_program(mode=mode)
    return _cached[key]


def _prep_perm_inputs(state_re, state_im):
    # S[j, 0:64] = state_re[:, j], S[j, 64:128] = state_im[:, j], bf16
    S = np.empty((N, P), dtype=bf16)
    S[:, :BATCH] = state_re.T
    S[:, BATCH:] = state_im.T
    # core c (X=c>>2, B=c&3) input shard: rows y*4096 + B*1024 + h*64 + X*32 + l
    Sv = S.reshape(2, 4, 16, 2, 4, 8, 128)  # [y, B, h, X, lh, ll, b]
    shards = []
    for c in range(NCORES):
        X, B = c >> 2, c & 3
        shards.append(np.ascontiguousarray(Sv[:, B, :, X]))  # [2,16,4,8,128]
    return shards


def _assemble_perm_output(res, out_name="out0"):
    # per-core out [16, 2, 4, 8, 128] -> rows j in [c*1024, (c+1)*1024)
    full = np.concatenate(
        [res.results[c][out_name].reshape(1024, P) for c in range(NCORES)], axis=0
    )  # [8192, 128] bf16
    fullT = full.astype(np.float32).T  # [128, 8192]
    return (fullT[:BATCH] + 1j * fullT[BATCH:]).astype(np.complex64)


# ---------------------------------------------------------------------------
# Fallback: dense matmul (column-sharded, fp16 hi/lo x fp8/bf16)
# ---------------------------------------------------------------------------

COLS = N // NCORES          # 1024 output columns per core
KT = N // P                 # 64 k-tiles
NCH = COLS // 512           # 2 psum chunks of 512
KBLK = 8                    # max k-tiles per M DMA block
BLOCKS = [2, 2, 4] + [8] * 7

SCALE_BITS = 22
SCALE = float(2 ** SCALE_BITS)
INV_SCALE = float(2.0 ** (-SCALE_BITS))


def _fp8_exact(M):
    sample = M[:: 64, :: 64]
    if not np.array_equal(sample.astype(f8e4).astype(np.float32), sample):
        return False
    return np.array_equal(M.astype(f8e4).astype(np.float32), M)


def _build_program(reps=1, serialize=False, m_dt="fp8"):
    import concourse.mybir as mybir
    import concourse.tile as tile
    from concourse import bacc

    mdt = {"fp8": mybir.dt.float8e4, "bf16": mybir.dt.bfloat16}[m_dt]
    nc = bacc.Bacc("TRN2", target_bir_lowering=False, debug=False)
    st_d = nc.declare_dram_parameter("st", [P, KT, 256], mybir.dt.float16, isOutput=False)
    m_d = nc.declare_dram_parameter("m", [P, KT, NCH, 512], mdt, isOutput=False)
    out_d = nc.declare_dram_parameter("out", [P, COLS], mybir.dt.float32, isOutput=True)

    with tile.TileContext(nc) as tc:
        with (
            tc.tile_pool(name="stp", bufs=1) as stp,
            tc.tile_pool(name="mp", bufs=4) as mp,
            tc.tile_pool(name="op", bufs=1) as op,
            tc.tile_pool(name="ps", bufs=1, space="PSUM") as ps,
        ):
            st_sb = stp.tile([P, KT, 256], mybir.dt.float16)
            k0 = 0
            for nb in BLOCKS:
                nc.sync.dma_start(st_sb[:, k0:k0 + nb, :], st_d[:, k0:k0 + nb, :])
                k0 += nb
            wsb = stp.tile([P, 128], mybir.dt.float16, name="wsb")
            nc.vector.memset(wsb[:], 0.0)
            wps = ps.tile([P, 128], mybir.dt.float32, name="wps")
            for _rep in range(reps):
                if serialize and reps > 1:
                    tc.strict_bb_all_engine_barrier()
                for _ in range(40):
                    nc.tensor.matmul(wps[:], wsb[:], wsb[:], start=True, stop=True)
                out_sb = op.tile([P, COLS], mybir.dt.float32, name="out_sb")
                ps_hi = [
                    ps.tile([P, 512], mybir.dt.float32, name=f"ps_hi{i}")
                    for i in range(NCH)
                ]
                ps_lo = [
                    ps.tile([P, 512], mybir.dt.float32, name=f"ps_lo{i}")
                    for i in range(NCH)
                ]
                k0 = 0
                for nb in BLOCKS:
                    m_sb = mp.tile([P, KBLK, NCH, 512], mdt, name="m_sb")
                    nc.sync.dma_start(m_sb[:, :nb], m_d[:, k0:k0 + nb, :, :])
                    for kj in range(nb):
                        ko = k0 + kj
                        for pss, c0 in ((ps_hi, 0), (ps_lo, 128)):
                            for nch in range(NCH):
                                nc.tensor.matmul(
                                    pss[nch][:],
                                    st_sb[:, ko, c0:c0 + 128],
                                    m_sb[:, kj, nch, :],
                                    start=(ko == 0),
                                    stop=(ko == KT - 1),
                                )
                    k0 += nb
                for nch in range(NCH):
                    sl = slice(nch * 512, (nch + 1) * 512)
                    nc.vector.tensor_scalar_mul(out_sb[:, sl], ps_lo[nch][:], INV_SCALE)
                    nc.vector.tensor_add(out_sb[:, sl], out_sb[:, sl], ps_hi[nch][:])
                nc.sync.dma_start(out_d[:], out_sb[:])
    nc.compile()
    return nc


def _get_program(m_dt="fp8"):
    key = f"nc_{m_dt}"
    if key not in _cached:
        _cached[key] = _build_program(m_dt=m_dt)
    return _cached[key]


def _prep_inputs(state_re, state_im, M, m_dt="fp8"):
    S = np.empty((N, P), dtype=np.float32)
    S[:, :BATCH] = state_re.T
    S[:, BATCH:] = state_im.T
    hi = S.astype(np.float16)
    lo = ((S - hi.astype(np.float32)) * SCALE).astype(np.float16)
    stall = np.concatenate([hi, lo], axis=1)  # [8192, 256] fp16
    st_tiled = np.ascontiguousarray(
        stall.reshape(KT, P, 256).transpose(1, 0, 2)
    )  # [128, 64, 256]

    Mb = M.astype(f8e4 if m_dt == "fp8" else bf16)
    m_tiles = []
    for c in range(NCORES):
        shard = Mb[:, c * COLS:(c + 1) * COLS]
        m_tiles.append(
            np.ascontiguousarray(
                shard.reshape(KT, P, NCH, 512).transpose(1, 0, 2, 3)
            )
        )  # [128, 64, 2, 512]
    return st_tiled, m_tiles


# ---------------------------------------------------------------------------
# Entry points
# ---------------------------------------------------------------------------

def run_on_hw(state_re, state_im, M, trace=False, force_dense=False,
              perm_mode="d2d"):
    from concourse.bass_utils import run_bass_kernel_spmd

    state_re = np.asarray(state_re, dtype=np.float32)
    state_im = np.asarray(state_im, dtype=np.float32)
    M = np.asarray(M, dtype=np.float32)

    trace_kw = dict(
        trace=trace, trace_cores=list(range(NCORES)) if trace else None
    )

    if not force_dense and _is_swap_matrix(M):
        nc = _get_perm_program(perm_mode)
        shards = _prep_perm_inputs(state_re, state_im)
        in_maps = [{"in": shards[c]} for c in range(NCORES)]
        res = run_bass_kernel_spmd(nc, in_maps, list(range(NCORES)), **trace_kw)
        return _assemble_perm_output(res), res

    m_dt = "fp8" if _fp8_exact(M) else "bf16"
    nc = _get_program(m_dt)
    st_tiled, m_tiles = _prep_inputs(state_re, state_im, M, m_dt)
    in_maps = [{"st": st_tiled, "m": m_tiles[c]} for c in range(NCORES)]
    res = run_bass_kernel_spmd(nc, in_maps, list(range(NCORES)), **trace_kw)
    full = np.concatenate([res.results[c]["out"] for c in range(NCORES)], axis=1)
    out = (full[:BATCH] + 1j * full[BATCH:]).astype(np.complex64)
    return out, res


def kernel(state_re, state_im, M):
    out, _ = run_on_hw(state_re, state_im, M, trace=False)
    return out
